# revision 2
# baseline (speedup 1.0000x reference)
"""Trainium2 Bass kernel for nn_CAM_50053548867817 (moe_routing mamba scan).

Wall-clock-oriented design: the graded metric is warm-call wall time, which
is dominated by the ~45 MB/s axon host<->device tunnel, so the kernel
minimizes wire bytes and pipelines everything else under them:
  host    : routing (one sgemm + argmax + stable argsort; x-norm is
            argmax-invariant so only means are normalized), one-hot prompt
            masks, int8 quantization of x with per-token f16 scales.
  wire up : ONE packed int8 tensor per core (~2.15 MB): L rows of
            [1024 int8 | f16 scale | pad] + 8 rows carrying sidx + 32 rows
            carrying the one-hot masks. Weight-derived constants are
            uploaded once and kept device-resident across calls.
  device  : indirect row gather (sorted order) -> int8 dequant via bitwise
            unpack + 2^23 magic-mantissa trick (DVE float->int converts are
            broken on this stack) -> bf16 xbar transposes -> x_proj/dt GEMMs
            + cluster-prompt add in PSUM -> softplus -> per-state selective
            scan (DVE tensor_tensor_scan) -> C-weighted tree reduction ->
            y -> f16 -> xbar transposes to token-major -> per-token int8
            quantize (max/min reduce + reciprocal + magic-add RNE rounding,
            byte-packed into i32 lanes) -> indirect row scatter back to
            original token order.
  wire dn : packed int8 y (~2.15 MB per core) decoded on host.
  dispatch: 8 cached fast-dispatch AOT executables (one per core) so
            quantize/upload/exec/download/decode pipeline across cores; no
            per-call retrace, no donated zero buffers (outputs are fully
            written on device).
Measured: ~0.80 s warm call vs 4.87 s baseline; rel err 1.34e-2 (gate 2e-2),
dominated by deliberate int8 wire quantization (f16 wire variant: 2.97e-3
at ~1.5 s).
"""

import os
import sys

os.environ.pop("BASS_TRACE", None)
os.environ["BASS_NEVER_TRACE"] = "1"

sys.path.insert(0, "/opt/trn_rl_repo")

import numpy as np
import ml_dtypes

import concourse.bass as bass
import concourse.bacc as bacc
import concourse.mybir as mybir
from concourse.tile import TileContext
from concourse.tile_rust import add_dep_helper
from concourse import bass2jax

F32 = mybir.dt.float32
F16 = mybir.dt.float16
BF16 = mybir.dt.bfloat16
I32 = mybir.dt.int32
AL = mybir.AluOpType
AF = mybir.ActivationFunctionType
AX = mybir.AxisListType
BF16NP = ml_dtypes.bfloat16

# problem shapes (hardcoded per contest rules)
B, L, DM, NS, DR, K = 8, 2048, 1024, 16, 32, 8
P = 128
NT = L // P          # 16 tau-tiles of 128 tokens
DB = DM // P         # 8 d-blocks
CH = 1024            # scan tau-chunk
NCH = L // CH        # 2
GC = 512             # GEMM/psum tau-chunk
NGC = L // GC        # 4
RW = DM // 4 + 2     # packed int8 row: 256 data words + scale word + pad
SX = 8               # extra rows carrying sidx (2048 i32 = 8 x 256 words)
SO = 32              # extra rows carrying ohs (8 x 2048 bf16 = 32 x 256 words)
XR = L + SX + SO     # total rows of the packed per-core input tensor
MAGIC = float(2 ** 23)
BIAS = MAGIC + 128.0


def build_program():
    nc = bacc.Bacc()

    # ---- DRAM I/O (declaration order == runner operand order) ----
    # x_q row: 256 i32 words of packed int8 (bias +128) + word 256 = f16
    # scale in low half + 1 pad word.  Same layout for y_q.
    x_q = nc.dram_tensor("x_q", (XR, RW), I32, kind="ExternalInput")
    cblob128 = nc.dram_tensor("cblob128", (P, 144), F32, kind="ExternalInput")
    cblobb = nc.dram_tensor("cblobb", (DR, 1168), BF16, kind="ExternalInput")
    wxpT = nc.dram_tensor("wxpT", (P, DB * 80), BF16, kind="ExternalInput")

    y_q = nc.dram_tensor("y_q", (L, RW), I32, kind="ExternalOutput")

    with TileContext(nc) as tc:
        with (
            tc.tile_pool(name="const", bufs=1) as cpool,
            tc.tile_pool(name="sidp", bufs=1) as sidp,
            tc.tile_pool(name="gath", bufs=2) as gp,
            tc.tile_pool(name="gathb", bufs=2) as gpb,
            tc.tile_pool(name="xsT", bufs=1) as xsTp,
            tc.tile_pool(name="mid", bufs=1) as midp,
            tc.tile_pool(name="rep", bufs=1) as repp,
            tc.tile_pool(name="scan", bufs=1) as scanp,
            tc.tile_pool(name="y16", bufs=1) as y16p,
            tc.tile_pool(name="ytd", bufs=1) as ytdp,
            tc.tile_pool(name="wrk", bufs=2) as wp,
            tc.tile_pool(name="wrk3", bufs=3) as wp3,
            tc.tile_pool(name="qtmp", bufs=1) as qp,
            tc.tile_pool(name="ps_big", bufs=2, space="PSUM") as psb,
            tc.tile_pool(name="ps_a", bufs=2, space="PSUM") as psa,
        ):
            # ---------- constants into SBUF ----------
            cb128 = cpool.tile([P, 144], F32, tag="cb128")
            nc.sync.dma_start(cb128[:], cblob128[:, :])
            cbb = cpool.tile([DR, 1168], BF16, tag="cbb")
            nc.sync.dma_start(cbb[:], cblobb[:, :])
            wxp_all = cpool.tile([P, DB * 80], BF16, tag="wxpa")
            nc.sync.dma_start(wxp_all[:], wxpT[:, :])
            ohs_i = cpool.tile([K, L // 2], I32, tag="ohsb")
            for k in range(K):
                for j in range(4):
                    nc.sync.dma_start(
                        ohs_i[k:k + 1, j * 256:(j + 1) * 256],
                        x_q[L + SX + 4 * k + j:L + SX + 4 * k + j + 1, 0:256])
            ohs_b = ohs_i[:].bitcast(BF16)
            ac_t = [cb128[:, d * NS:(d + 1) * NS] for d in range(DB)]
            ds_t = [cb128[:, 128 + d:129 + d] for d in range(DB)]
            dtb_t = [cb128[:, 136 + d:137 + d] for d in range(DB)]
            wdt = cbb[:, 0:DM]
            cpr = cbb[0:K, DM:DM + NS]
            onrb = cbb[0:1, DM + NS:DM + NS + P]
            wxp_t = [wxp_all[:, d * 80:(d + 1) * 80] for d in range(DB)]

            # ---------- stage A: load sidx, gather rows, cast, transpose ----
            sid_t = []
            for t in range(NT):
                sid = sidp.tile([P, 1], I32, tag=f"sid{t}")
                r = L + t // 2
                c0 = (t % 2) * P
                nc.sync.dma_start(
                    sid[:], x_q[r:r + 1, c0:c0 + P].rearrange("a b -> b a"))
                sid_t.append(sid)
            tr_prev = [None] * DB
            xsT_t = []
            for d in range(DB):
                xt = xsTp.tile([P, L], BF16, tag=f"xsT{d}")
                xsT_t.append(xt)
            for t in range(NT):
                grow32 = gp.tile([P, RW], I32, tag="grow32")
                nc.gpsimd.indirect_dma_start(
                    out=grow32[:],
                    out_offset=None,
                    in_=x_q[0:L, :],
                    in_offset=bass.IndirectOffsetOnAxis(ap=sid_t[t][:, :1], axis=0),
                    bounds_check=L - 1,
                    oob_is_err=False,
                )
                # per-row f16 scale at byte offset 1024 (f16 slot 512)
                sc = wp.tile([P, 1], F32, tag="gsc")
                nc.vector.tensor_copy(
                    sc[:], grow32[:].bitcast(F16)[:, 2 * (RW - 2):2 * (RW - 2) + 1])
                # unpack bytes k::4 via magic-mantissa trick, dequant to bf16
                growb = gpb.tile([P, DM], BF16, tag="growb")
                for kk in range(4):
                    sh = qp.tile([P, DM // 4], I32, tag="unp_sh")
                    if kk:
                        nc.vector.tensor_scalar(
                            out=sh[:], in0=grow32[:, 0:DM // 4], scalar1=8 * kk,
                            scalar2=255, op0=AL.logical_shift_right,
                            op1=AL.bitwise_and)
                    else:
                        nc.vector.tensor_scalar(
                            out=sh[:], in0=grow32[:, 0:DM // 4], scalar1=255,
                            scalar2=None, op0=AL.bitwise_and)
                    nc.vector.tensor_scalar(out=sh[:], in0=sh[:],
                                            scalar1=0x4B000000, scalar2=None,
                                            op0=AL.bitwise_or)
                    nc.vector.tensor_scalar(
                        out=growb[:, kk::4], in0=sh[:].bitcast(F32),
                        scalar1=BIAS, scalar2=sc[:, :1],
                        op0=AL.subtract, op1=AL.mult)
                for d in range(DB):
                    tr = nc.sync.dma_start_transpose(
                        out=xsT_t[d][:, t * P:(t + 1) * P],
                        in_=growb[:, d * P:(d + 1) * P],
                    )
                    if tr_prev[d] is not None:
                        add_dep_helper(tr.ins, tr_prev[d].ins, True, "tr chain")
                    tr_prev[d] = tr

            # ---------- stage B: x_proj GEMM + prompt, per GC chunk ----------
            dts_b = midp.tile([DR, L], BF16, tag="dtsb")
            bm_b = midp.tile([NS, L], BF16, tag="bmb")
            cm_b = midp.tile([NS, L], BF16, tag="cmb")
            for c in range(NGC):
                sl = slice(c * GC, (c + 1) * GC)
                psx = psb.tile([80, GC], F32, tag="psbig")
                for d in range(DB):
                    nc.tensor.matmul(out=psx[:], lhsT=wxp_t[d][:],
                                     rhs=xsT_t[d][:, sl],
                                     start=(d == 0), stop=False)
                # wxpT columns are host-reordered to [dts | Cm | Bm] so the
                # prompt add lands at PSUM base partition 32 (HW constraint).
                nc.tensor.matmul(out=psx[32:48, :], lhsT=cpr[:], rhs=ohs_b[:, sl],
                                 start=False, stop=True)
                nc.scalar.activation(dts_b[:, sl], psx[0:DR, :], AF.Copy)
                nc.scalar.activation(cm_b[:, sl], psx[32:48, :], AF.Copy)
                nc.scalar.activation(bm_b[:, sl], psx[64:80, :], AF.Copy)

            # ---------- stage C: scan over chunks + output ----------
            hlast = []
            for d in range(DB):
                hl = cpool.tile([P, NS], F32, tag=f"hl{d}")
                hlast.append(hl)
            scat_prev = None

            for c2 in range(NCH):
                csl = slice(c2 * CH, (c2 + 1) * CH)
                # build replicated B/C (128, NS*CH) bf16 via K=1 matmul + ACT copy
                brep = repp.tile([P, NS * CH], BF16, tag="brep")
                crep = repp.tile([P, NS * CH], BF16, tag="crep")
                for n in range(NS):
                    for src_t, dst_t, tg in ((bm_b, brep, "brow"),
                                             (cm_b, crep, "crow")):
                        row0 = wp.tile([1, CH], BF16, tag=tg)
                        nc.sync.dma_start(row0[:], src_t[n:n + 1, csl])
                        for h in range(CH // GC):
                            pr = psb.tile([P, GC], F32, tag="psbig")
                            nc.tensor.matmul(
                                out=pr[:], lhsT=onrb[:],
                                rhs=row0[:, h * GC:(h + 1) * GC],
                                start=True, stop=True)
                            nc.scalar.activation(
                                dst_t[:, n * CH + h * GC:n * CH + (h + 1) * GC],
                                pr[:], AF.Copy)

                y16_all = y16p.tile([P, DB * CH], F16, tag="y16")
                for d in range(DB):
                    # delta via dt GEMM + softplus (per GC for psum limit)
                    delta = wp.tile([P, CH], F32, tag="delta")
                    for h in range(CH // GC):
                        s_src = slice(c2 * CH + h * GC, c2 * CH + (h + 1) * GC)
                        s_dst = slice(h * GC, (h + 1) * GC)
                        psd = psb.tile([P, GC], F32, tag="psbig")
                        nc.tensor.matmul(out=psd[:],
                                         lhsT=wdt[:, d * P:(d + 1) * P],
                                         rhs=dts_b[:, s_src],
                                         start=True, stop=True)
                        # softplus(x) = ln(exp(x) + 1); Exp/Ln share one table set
                        esp = psb.tile([P, GC], F32, tag="psbig", space="PSUM")
                        nc.scalar.activation(esp[:], psd[:], AF.Exp,
                                             bias=dtb_t[d][:, :1], scale=1.0)
                        nc.scalar.activation(delta[:, s_dst], esp[:], AF.Ln,
                                             bias=1.0, scale=1.0)
                    du = wp.tile([P, CH], BF16, tag="du")
                    nc.vector.tensor_tensor(out=du[:], in0=delta[:],
                                            in1=xsT_t[d][:, csl], op=AL.mult)

                    h_all = scanp.tile([P, NS * CH], BF16, tag="h_all")
                    for n in range(NS):
                        nsl = slice(n * CH, (n + 1) * CH)
                        a_ps = psa.tile([P, CH], F32, tag="a_ps")
                        nc.scalar.activation(a_ps[:], delta[:], AF.Exp,
                                             scale=ac_t[d][:, n:n + 1])
                        b_sb = wp3.tile([P, CH], BF16, tag="b_sb")
                        nc.vector.tensor_tensor(out=b_sb[:], in0=du[:],
                                                in1=brep[:, nsl], op=AL.mult)
                        init = 0.0 if c2 == 0 else hlast[d][:, n:n + 1]
                        nc.vector.tensor_tensor_scan(
                            out=h_all[:, nsl], data0=a_ps[:], data1=b_sb[:],
                            initial=init, op0=AL.mult, op1=AL.add)
                    # save last state (strided copy) BEFORE overwriting h_all
                    if c2 + 1 < NCH:
                        nc.vector.tensor_copy(
                            hlast[d][:, :],
                            h_all[:, CH - 1::CH])
                    # y = sum_n C_n * h_n  (in-place mult then tree halving)
                    nc.vector.tensor_tensor(out=h_all[:], in0=h_all[:],
                                            in1=crep[:], op=AL.mult)
                    width = NS * CH // 2
                    while width >= CH:
                        nc.vector.tensor_tensor(
                            out=h_all[:, 0:width],
                            in0=h_all[:, 0:width],
                            in1=h_all[:, width:2 * width], op=AL.add)
                        width //= 2
                    # y in f16 straight into the per-chunk staging tile
                    nc.vector.scalar_tensor_tensor(
                        out=y16_all[:, d * CH:(d + 1) * CH],
                        in0=xsT_t[d][:, csl],
                        scalar=ds_t[d][:, :1], in1=h_all[:, 0:CH],
                        op0=AL.mult, op1=AL.add)

                # ---- output: transpose to token-major, int8-quantize, scatter
                for tt in range(CH // P):
                    ytd = ytdp.tile([P, DM], F16, tag="ytd")
                    tprev = None
                    for d in range(DB):
                        tr = nc.sync.dma_start_transpose(
                            out=ytd[:, d * P:(d + 1) * P],
                            in_=y16_all[:, d * CH + tt * P:d * CH + (tt + 1) * P],
                        )
                        if tprev is not None:
                            add_dep_helper(tr.ins, tprev.ins, True, "ytr chain")
                        tprev = tr
                    # per-token scale, quantize via magic add, pack bytes
                    mx = wp.tile([P, 1], F32, tag="qmx")
                    nc.vector.tensor_reduce(mx[:], ytd[:], axis=AX.X,
                                            op=AL.max)
                    mn = wp.tile([P, 1], F32, tag="qmn")
                    nc.vector.tensor_reduce(mn[:], ytd[:], axis=AX.X,
                                            op=AL.min)
                    nc.vector.tensor_scalar(out=mn[:], in0=mn[:],
                                            scalar1=-1.0, scalar2=None,
                                            op0=AL.mult)
                    nc.vector.tensor_tensor(out=mx[:], in0=mx[:], in1=mn[:],
                                            op=AL.max)
                    inv = wp.tile([P, 1], F32, tag="qinv")
                    nc.vector.reciprocal(inv[:], mx[:])
                    inv127 = wp.tile([P, 1], F32, tag="qinv127")
                    nc.vector.tensor_scalar(out=inv127[:], in0=inv[:],
                                            scalar1=127.0, scalar2=None,
                                            op0=AL.mult)
                    t32q = ytdp.tile([P, DM], F32, tag="t32q")
                    nc.vector.tensor_scalar(out=t32q[:], in0=ytd[:],
                                            scalar1=inv127[:, :1], scalar2=BIAS,
                                            op0=AL.mult, op1=AL.add)
                    qpk = ytdp.tile([P, RW], I32, tag="qpk")
                    for kk in range(4):
                        lane = qp.tile([P, DM // 4], F32, tag="pk_lane")
                        nc.vector.tensor_copy(lane[:], t32q[:, kk::4])
                        li = lane[:].bitcast(I32)
                        if kk:
                            nc.vector.tensor_scalar(
                                out=li, in0=li, scalar1=255,
                                scalar2=8 * kk, op0=AL.bitwise_and,
                                op1=AL.logical_shift_left)
                            nc.vector.tensor_tensor(
                                out=qpk[:, 0:DM // 4], in0=qpk[:, 0:DM // 4],
                                in1=li, op=AL.bitwise_or)
                        else:
                            nc.vector.tensor_scalar(
                                out=qpk[:, 0:DM // 4], in0=li,
                                scalar1=255, scalar2=None, op0=AL.bitwise_and)
                    # f16 dequant scale into byte offset 1024
                    nc.vector.tensor_scalar(
                        out=qpk[:].bitcast(F16)[:, 2 * (RW - 2):2 * (RW - 2) + 1],
                        in0=mx[:], scalar1=1.0 / 127.0, scalar2=None,
                        op0=AL.mult)
                    sc = nc.gpsimd.indirect_dma_start(
                        out=y_q[:, :],
                        out_offset=bass.IndirectOffsetOnAxis(
                            ap=sid_t[c2 * (CH // P) + tt][:, :1], axis=0),
                        in_=qpk[:],
                        in_offset=None,
                        bounds_check=L - 1,
                        oob_is_err=False,
                    )
                    if scat_prev is not None:
                        add_dep_helper(sc.ins, scat_prev.ins, True, "scat chain")
                    scat_prev = sc
    nc.compile()
    return nc


_EPS = 1e-12


def _pack_consts(means, prompt_weight, x_proj_weight, dt_projs_weight,
                 dt_projs_bias, A_logs, Ds):
    cluster_prompts = means @ prompt_weight.T          # (K, NS)
    A = -np.exp(A_logs)                                # (DM, NS)
    cb128 = np.zeros((P, 144), np.float32)
    for d in range(DB):
        cb128[:, d * NS:(d + 1) * NS] = A[d * P:(d + 1) * P, :]
        cb128[:, 128 + d] = Ds[d * P:(d + 1) * P]
        cb128[:, 136 + d] = dt_projs_bias[d * P:(d + 1) * P]
    cbb = np.zeros((DR, 1168), np.float32)
    cbb[:, 0:DM] = dt_projs_weight.T
    cbb[0:K, DM:DM + NS] = cluster_prompts
    cbb[0, DM + NS:DM + NS + P] = 1.0
    wxp80 = np.concatenate([
        x_proj_weight[0:DR],                     # dts rows 0:32
        x_proj_weight[DR + NS:DR + 2 * NS],      # Cm rows 32:48
        np.zeros((NS, DM), np.float32),          # pad rows 48:64
        x_proj_weight[DR:DR + NS],               # Bm rows 64:80
    ], axis=0).T                                 # (DM, 80)
    return {
        "cblob128": cb128,
        "cblobb": cbb.astype(BF16NP),
        "wxpT": np.ascontiguousarray(
            wxp80.reshape(DB, P, 80).transpose(1, 0, 2).reshape(P, DB * 80)
        ).astype(BF16NP),
    }


NW = 8               # waves
WC = B // NW         # cores (= batch rows) per wave


class _Runtime:
    def __init__(self):
        import jax
        from jax.sharding import Mesh, PartitionSpec, NamedSharding
        from jax.experimental.shard_map import shard_map

        self.jax = jax
        nc = build_program()
        self.nc = nc
        bass2jax.install_neuronx_cc_hook()

        partition_name = (nc.partition_id_tensor.name
                          if nc.partition_id_tensor else None)
        in_names, in_shapes, in_dtypes = [], [], []
        out_names, out_avals = [], []
        for alloc in nc.m.functions[0].allocations:
            if not isinstance(alloc, mybir.MemoryLocationSet):
                continue
            name = alloc.memorylocations[0].name
            if alloc.kind == "ExternalInput":
                if name != partition_name:
                    in_names.append(name)
                    in_shapes.append(tuple(alloc.tensor_shape))
                    in_dtypes.append(mybir.dt.np(alloc.dtype))
            elif alloc.kind == "ExternalOutput":
                out_names.append(name)
                out_avals.append(jax.core.ShapedArray(
                    tuple(alloc.tensor_shape), mybir.dt.np(alloc.dtype)))
        self.in_names = in_names
        names_cfg = list(in_names) + ([partition_name] if partition_name else [])

        def _body(*args):
            operands = list(args)
            if partition_name is not None:
                operands.append(bass2jax.partition_id_tensor())
            outs = bass2jax._bass_exec_p.bind(
                *operands,
                out_avals=tuple(out_avals),
                in_names=tuple(names_cfg),
                out_names=tuple(out_names),
                lowering_input_output_aliases=(),
                sim_require_finite=True,
                sim_require_nnan=True,
                nc=nc,
            )
            return tuple(outs)

        devices = jax.devices()[:B]
        spec = PartitionSpec("core")
        self.sh_w = []
        self.compiled_w = []
        for w in range(NW):
            mesh = Mesh(np.asarray(devices[w * WC:(w + 1) * WC]), ("core",))
            sh = NamedSharding(mesh, spec)
            self.sh_w.append(sh)
            fn = shard_map(_body, mesh=mesh,
                           in_specs=(spec,) * len(in_names),
                           out_specs=spec,
                           check_rep=False)

            def wrapped(*args, _fn=fn):
                return _fn(*args)[0]
            structs = [
                jax.ShapeDtypeStruct((WC * s[0],) + s[1:], d, sharding=sh)
                for s, d in zip(in_shapes, in_dtypes)
            ]
            self.compiled_w.append(bass2jax.fast_dispatch_compile(
                lambda: jax.jit(wrapped).lower(*structs).compile()))
        self.const_key = None
        self.const_dev = None
        dummy_w = [
            [self.put(np.zeros((WC * s[0],) + s[1:], d), w)
             for s, d in zip(in_shapes, in_dtypes)]
            for w in range(NW)
        ]
        for w in range(NW):
            jax.block_until_ready(self.compiled_w[w](*dummy_w[w]))

    def put(self, arr, w):
        return self.jax.device_put(arr, self.sh_w[w])

    def set_consts(self, weights):
        key = tuple(np.asarray(w).tobytes() for w in weights)
        if self.const_key is not None and key == self.const_key:
            return
        consts = _pack_consts(*[np.asarray(w, np.float32) for w in weights])
        self.const_dev = [
            [self.put(np.ascontiguousarray(np.tile(consts[n], (WC, 1))), w)
             for n in ("cblob128", "cblobb", "wxpT")]
            for w in range(NW)
        ]
        self.jax.block_until_ready(self.const_dev)
        self.const_key = key


_RT = None


def _runtime():
    global _RT
    if _RT is None:
        _RT = _Runtime()
    return _RT


def kernel(x, means, prompt_weight, x_proj_weight, dt_projs_weight,
           dt_projs_bias, A_logs, Ds):
    try:
        return _kernel(x, means, prompt_weight, x_proj_weight,
                       dt_projs_weight, dt_projs_bias, A_logs, Ds)
    except Exception:
        import time
        time.sleep(2.0)
        return _kernel(x, means, prompt_weight, x_proj_weight,
                       dt_projs_weight, dt_projs_bias, A_logs, Ds)


def _kernel(x, means, prompt_weight, x_proj_weight, dt_projs_weight,
            dt_projs_bias, A_logs, Ds):
    x = np.asarray(x, np.float32)
    rt = _runtime()
    rt.set_consts((means, prompt_weight, x_proj_weight, dt_projs_weight,
                   dt_projs_bias, A_logs, Ds))

    # ---- host routing: argmax over cosine scores (x-norm is k-invariant) ----
    means = np.asarray(means, np.float32)
    mnorm = means / np.maximum(
        np.linalg.norm(means, axis=-1, keepdims=True), _EPS)
    scores = x.reshape(B * L, DM) @ mnorm.T            # (B*L, K) f32 sgemm
    buckets = np.argmax(scores, axis=1).reshape(B, L)
    sidx = np.argsort(buckets, axis=1, kind="stable").astype(np.int32)  # (B,L)
    b_sorted = np.take_along_axis(buckets, sidx, axis=1)                # (B,L)
    ohs = (b_sorted[:, None, :] == np.arange(K)[None, :, None])
    ohs = ohs.astype(np.float32).astype(BF16NP)        # (B,K,L)

    # ---- wave pipeline: quantize+pack -> upload -> exec -> fetch+decode ----
    import concurrent.futures as cf
    xr = x.reshape(B * L, DM)
    y = np.empty((B, L, DM), np.float32)

    def quant_and_put(w):
        # one wave == one core == one batch row; single packed upload
        rows = slice(w * WC * L, (w + 1) * WC * L)
        xw = xr[rows]
        mx = np.maximum(xw.max(axis=1), -xw.min(axis=1))
        scl16 = (mx * (1.0 / 127.0)).astype(np.float16)
        inv127 = 1.0 / scl16.astype(np.float32)
        xq = np.empty((WC * XR, 4 * RW), np.uint8)
        xq3 = xq.reshape(WC, XR, 4 * RW)
        q = xw * inv127[:, None]
        np.rint(q, out=q)
        q += 128.0
        for c in range(WC):
            b = w * WC + c
            xq3[c, 0:L, 0:DM] = q[c * L:(c + 1) * L]
            xq3[c, 0:L, DM:DM + 2] = (
                scl16[c * L:(c + 1) * L].view(np.uint16)[:, None].view(np.uint8))
            xq3[c, 0:L, DM + 2:] = 0
            xq3[c, L:L + SX, 0:1024] = (
                sidx[b].view(np.uint8).reshape(SX, 1024))
            xq3[c, L + SX:XR, 0:1024] = (
                np.ascontiguousarray(ohs[b]).view(np.uint8).reshape(SO, 1024))
            xq3[c, L:XR, 1024:] = 0
        return rt.put(xq.view(np.int32), w)

    def fetch_decode(w, out):
        yq = np.asarray(out).view(np.uint8)             # (WC*L, 4*RW) bytes
        yv = yq[:, 0:DM].astype(np.float32)             # biased bytes (q+128)
        yv -= 128.0
        ysc = yq[:, DM:DM + 2].copy().view(np.float16).astype(np.float32)
        yv *= ysc
        y[w * WC:(w + 1) * WC] = yv.reshape(WC, L, DM)

    with cf.ThreadPoolExecutor(2 * NW) as ex:
        put_futs = [ex.submit(quant_and_put, w) for w in range(NW)]
        fetch_futs = []
        for w, f in enumerate(put_futs):
            out = rt.compiled_w[w](f.result(), *rt.const_dev[w])
            fetch_futs.append(ex.submit(fetch_decode, w, out))
        for f in fetch_futs:
            f.result()
    return y


if __name__ == "__main__":
    np.random.seed(0)
    ins = {
        "x": np.random.randn(B, L, DM).astype(np.float32),
        "means": np.random.randn(K, DM).astype(np.float32),
        "prompt_weight": np.random.randn(NS, DM).astype(np.float32) * DM ** -0.5,
        "x_proj_weight": np.random.randn(DR + 2 * NS, DM).astype(np.float32) * DM ** -0.5,
        "dt_projs_weight": np.random.uniform(-DR ** -0.5, DR ** -0.5, (DM, DR)).astype(np.float32),
        "dt_projs_bias": np.random.randn(DM).astype(np.float32),
        "A_logs": np.log(np.broadcast_to(np.arange(1, NS + 1, dtype=np.float32), (DM, NS))).copy(),
        "Ds": np.ones(DM, np.float32),
    }
    import time
    o = kernel(**ins)
    t0 = time.time()
    o = kernel(**ins)
    print(f"warm call: {time.time()-t0:.2f}s")
    print("ok", o.shape, o.dtype)


# revision 3
# speedup vs baseline: 1.0288x; 1.0288x over previous
"""Trainium2 Bass kernel for nn_CAM_50053548867817 (moe_routing mamba scan).

Wall-clock-oriented design: the graded metric is warm-call wall time, which
is dominated by the ~45 MB/s axon host<->device tunnel, so the kernel
minimizes wire bytes and pipelines everything else under them:
  host    : routing (one sgemm + argmax + stable argsort; x-norm is
            argmax-invariant so only means are normalized), one-hot prompt
            masks, int8 quantization of x with per-token f16 scales.
  wire up : ONE packed int8 tensor per core (~2.15 MB): L rows of
            [1024 int8 | f16 scale | pad] + 8 rows carrying sidx + 32 rows
            carrying the one-hot masks. Weight-derived constants are
            uploaded once and kept device-resident across calls.
  device  : indirect row gather (sorted order) -> int8 dequant via bitwise
            unpack + 2^23 magic-mantissa trick (DVE float->int converts are
            broken on this stack) -> bf16 xbar transposes -> x_proj/dt GEMMs
            + cluster-prompt add in PSUM -> softplus -> per-state selective
            scan (DVE tensor_tensor_scan) -> C-weighted tree reduction ->
            y -> f16 -> xbar transposes to token-major -> per-token int8
            quantize (max/min reduce + reciprocal + magic-add RNE rounding,
            byte-packed into i32 lanes) -> indirect row scatter back to
            original token order.
  wire dn : packed int8 y (~2.15 MB per core) decoded on host.
  dispatch: 8 cached fast-dispatch AOT executables (one per core) so
            quantize/upload/exec/download/decode pipeline across cores; no
            per-call retrace, no donated zero buffers (outputs are fully
            written on device).
Measured: ~0.80 s warm call vs 4.87 s baseline; rel err 1.34e-2 (gate 2e-2),
dominated by deliberate int8 wire quantization (f16 wire variant: 2.97e-3
at ~1.5 s).
"""

import os
import sys

os.environ.pop("BASS_TRACE", None)
os.environ["BASS_NEVER_TRACE"] = "1"

sys.path.insert(0, "/opt/trn_rl_repo")

import numpy as np
import ml_dtypes

import concourse.bass as bass
import concourse.bacc as bacc
import concourse.mybir as mybir
from concourse.tile import TileContext
from concourse.tile_rust import add_dep_helper
from concourse import bass2jax

F32 = mybir.dt.float32
F16 = mybir.dt.float16
BF16 = mybir.dt.bfloat16
I32 = mybir.dt.int32
AL = mybir.AluOpType
AF = mybir.ActivationFunctionType
AX = mybir.AxisListType
BF16NP = ml_dtypes.bfloat16

# problem shapes (hardcoded per contest rules)
B, L, DM, NS, DR, K = 8, 2048, 1024, 16, 32, 8
P = 128
NT = L // P          # 16 tau-tiles of 128 tokens
DB = DM // P         # 8 d-blocks
CH = 1024            # scan tau-chunk
NCH = L // CH        # 2
GC = 512             # GEMM/psum tau-chunk
NGC = L // GC        # 4
RW = DM // 4 + 2     # packed int8 row: 256 data words + scale word + pad
SX = 8               # extra rows carrying sidx (2048 i32 = 8 x 256 words)
SO = 32              # extra rows carrying ohs (8 x 2048 bf16 = 32 x 256 words)
XR = L + SX + SO     # total rows of the packed per-core input tensor
MAGIC = float(2 ** 23)
BIAS = MAGIC + 128.0


def build_program():
    nc = bacc.Bacc()

    # ---- DRAM I/O (declaration order == runner operand order) ----
    # x_q row: 256 i32 words of packed int8 (bias +128) + word 256 = f16
    # scale in low half + 1 pad word.  Same layout for y_q.
    x_q = nc.dram_tensor("x_q", (XR, RW), I32, kind="ExternalInput")
    cblob128 = nc.dram_tensor("cblob128", (P, 144), F32, kind="ExternalInput")
    cblobb = nc.dram_tensor("cblobb", (DR, 1168), BF16, kind="ExternalInput")
    wxpT = nc.dram_tensor("wxpT", (P, DB * 80), BF16, kind="ExternalInput")

    y_q = nc.dram_tensor("y_q", (L, RW), I32, kind="ExternalOutput")

    with TileContext(nc) as tc:
        with (
            tc.tile_pool(name="const", bufs=1) as cpool,
            tc.tile_pool(name="sidp", bufs=1) as sidp,
            tc.tile_pool(name="gath", bufs=2) as gp,
            tc.tile_pool(name="gathb", bufs=2) as gpb,
            tc.tile_pool(name="xsT", bufs=1) as xsTp,
            tc.tile_pool(name="mid", bufs=1) as midp,
            tc.tile_pool(name="rep", bufs=1) as repp,
            tc.tile_pool(name="scan", bufs=1) as scanp,
            tc.tile_pool(name="y16", bufs=1) as y16p,
            tc.tile_pool(name="ytd", bufs=1) as ytdp,
            tc.tile_pool(name="wrk", bufs=2) as wp,
            tc.tile_pool(name="wrk3", bufs=3) as wp3,
            tc.tile_pool(name="qtmp", bufs=1) as qp,
            tc.tile_pool(name="ps_big", bufs=2, space="PSUM") as psb,
            tc.tile_pool(name="ps_a", bufs=2, space="PSUM") as psa,
        ):
            # ---------- constants into SBUF ----------
            cb128 = cpool.tile([P, 144], F32, tag="cb128")
            nc.sync.dma_start(cb128[:], cblob128[:, :])
            cbb = cpool.tile([DR, 1168], BF16, tag="cbb")
            nc.sync.dma_start(cbb[:], cblobb[:, :])
            wxp_all = cpool.tile([P, DB * 80], BF16, tag="wxpa")
            nc.sync.dma_start(wxp_all[:], wxpT[:, :])
            ohs_i = cpool.tile([K, L // 2], I32, tag="ohsb")
            for k in range(K):
                for j in range(4):
                    nc.sync.dma_start(
                        ohs_i[k:k + 1, j * 256:(j + 1) * 256],
                        x_q[L + SX + 4 * k + j:L + SX + 4 * k + j + 1, 0:256])
            ohs_b = ohs_i[:].bitcast(BF16)
            ac_t = [cb128[:, d * NS:(d + 1) * NS] for d in range(DB)]
            ds_t = [cb128[:, 128 + d:129 + d] for d in range(DB)]
            dtb_t = [cb128[:, 136 + d:137 + d] for d in range(DB)]
            wdt = cbb[:, 0:DM]
            cpr = cbb[0:K, DM:DM + NS]
            onrb = cbb[0:1, DM + NS:DM + NS + P]
            wxp_t = [wxp_all[:, d * 80:(d + 1) * 80] for d in range(DB)]

            # ---------- stage A: load sidx, gather rows, cast, transpose ----
            sid_t = []
            for t in range(NT):
                sid = sidp.tile([P, 1], I32, tag=f"sid{t}")
                r = L + t // 2
                c0 = (t % 2) * P
                nc.sync.dma_start(
                    sid[:], x_q[r:r + 1, c0:c0 + P].rearrange("a b -> b a"))
                sid_t.append(sid)
            tr_prev = [None] * DB
            xsT_t = []
            for d in range(DB):
                xt = xsTp.tile([P, L], BF16, tag=f"xsT{d}")
                xsT_t.append(xt)
            for t in range(NT):
                grow32 = gp.tile([P, RW], I32, tag="grow32")
                nc.gpsimd.indirect_dma_start(
                    out=grow32[:],
                    out_offset=None,
                    in_=x_q[0:L, :],
                    in_offset=bass.IndirectOffsetOnAxis(ap=sid_t[t][:, :1], axis=0),
                    bounds_check=L - 1,
                    oob_is_err=False,
                )
                # per-row f16 scale at byte offset 1024 (f16 slot 512)
                sc = wp.tile([P, 1], F32, tag="gsc")
                nc.vector.tensor_copy(
                    sc[:], grow32[:].bitcast(F16)[:, 2 * (RW - 2):2 * (RW - 2) + 1])
                # unpack bytes k::4 via magic-mantissa trick, dequant to bf16
                growb = gpb.tile([P, DM], BF16, tag="growb")
                for kk in range(4):
                    sh = qp.tile([P, DM // 4], I32, tag="unp_sh")
                    if kk:
                        nc.vector.tensor_scalar(
                            out=sh[:], in0=grow32[:, 0:DM // 4], scalar1=8 * kk,
                            scalar2=255, op0=AL.logical_shift_right,
                            op1=AL.bitwise_and)
                    else:
                        nc.vector.tensor_scalar(
                            out=sh[:], in0=grow32[:, 0:DM // 4], scalar1=255,
                            scalar2=None, op0=AL.bitwise_and)
                    nc.vector.tensor_scalar(out=sh[:], in0=sh[:],
                                            scalar1=0x4B000000, scalar2=None,
                                            op0=AL.bitwise_or)
                    nc.vector.tensor_scalar(
                        out=growb[:, kk::4], in0=sh[:].bitcast(F32),
                        scalar1=BIAS, scalar2=sc[:, :1],
                        op0=AL.subtract, op1=AL.mult)
                for d in range(DB):
                    tr = nc.sync.dma_start_transpose(
                        out=xsT_t[d][:, t * P:(t + 1) * P],
                        in_=growb[:, d * P:(d + 1) * P],
                    )
                    if tr_prev[d] is not None:
                        add_dep_helper(tr.ins, tr_prev[d].ins, True, "tr chain")
                    tr_prev[d] = tr

            # ---------- stage B: x_proj GEMM + prompt, per GC chunk ----------
            dts_b = midp.tile([DR, L], BF16, tag="dtsb")
            bm_b = midp.tile([NS, L], BF16, tag="bmb")
            cm_b = midp.tile([NS, L], BF16, tag="cmb")
            for c in range(NGC):
                sl = slice(c * GC, (c + 1) * GC)
                psx = psb.tile([80, GC], F32, tag="psbig")
                for d in range(DB):
                    nc.tensor.matmul(out=psx[:], lhsT=wxp_t[d][:],
                                     rhs=xsT_t[d][:, sl],
                                     start=(d == 0), stop=False)
                # wxpT columns are host-reordered to [dts | Cm | Bm] so the
                # prompt add lands at PSUM base partition 32 (HW constraint).
                nc.tensor.matmul(out=psx[32:48, :], lhsT=cpr[:], rhs=ohs_b[:, sl],
                                 start=False, stop=True)
                nc.scalar.activation(dts_b[:, sl], psx[0:DR, :], AF.Copy)
                nc.scalar.activation(cm_b[:, sl], psx[32:48, :], AF.Copy)
                nc.scalar.activation(bm_b[:, sl], psx[64:80, :], AF.Copy)

            # ---------- stage C: scan over chunks + output ----------
            hlast = []
            for d in range(DB):
                hl = cpool.tile([P, NS], F32, tag=f"hl{d}")
                hlast.append(hl)
            scat_prev = None

            for c2 in range(NCH):
                csl = slice(c2 * CH, (c2 + 1) * CH)
                # build replicated B/C (128, NS*CH) bf16 via K=1 matmul + ACT copy
                brep = repp.tile([P, NS * CH], BF16, tag="brep")
                crep = repp.tile([P, NS * CH], BF16, tag="crep")
                for n in range(NS):
                    for src_t, dst_t, tg in ((bm_b, brep, "brow"),
                                             (cm_b, crep, "crow")):
                        row0 = wp.tile([1, CH], BF16, tag=tg)
                        nc.sync.dma_start(row0[:], src_t[n:n + 1, csl])
                        for h in range(CH // GC):
                            pr = psb.tile([P, GC], F32, tag="psbig")
                            nc.tensor.matmul(
                                out=pr[:], lhsT=onrb[:],
                                rhs=row0[:, h * GC:(h + 1) * GC],
                                start=True, stop=True)
                            nc.scalar.activation(
                                dst_t[:, n * CH + h * GC:n * CH + (h + 1) * GC],
                                pr[:], AF.Copy)

                y16_all = y16p.tile([P, DB * CH], F16, tag="y16")
                for d in range(DB):
                    # delta via dt GEMM + softplus (per GC for psum limit)
                    delta = wp.tile([P, CH], F32, tag="delta")
                    for h in range(CH // GC):
                        s_src = slice(c2 * CH + h * GC, c2 * CH + (h + 1) * GC)
                        s_dst = slice(h * GC, (h + 1) * GC)
                        psd = psb.tile([P, GC], F32, tag="psbig")
                        nc.tensor.matmul(out=psd[:],
                                         lhsT=wdt[:, d * P:(d + 1) * P],
                                         rhs=dts_b[:, s_src],
                                         start=True, stop=True)
                        # softplus(x) = ln(exp(x) + 1); Exp/Ln share one table set
                        esp = psb.tile([P, GC], F32, tag="psbig", space="PSUM")
                        nc.scalar.activation(esp[:], psd[:], AF.Exp,
                                             bias=dtb_t[d][:, :1], scale=1.0)
                        nc.scalar.activation(delta[:, s_dst], esp[:], AF.Ln,
                                             bias=1.0, scale=1.0)
                    du = wp.tile([P, CH], BF16, tag="du")
                    nc.vector.tensor_tensor(out=du[:], in0=delta[:],
                                            in1=xsT_t[d][:, csl], op=AL.mult)

                    h_all = scanp.tile([P, NS * CH], BF16, tag="h_all")
                    for n in range(NS):
                        nsl = slice(n * CH, (n + 1) * CH)
                        a_ps = psa.tile([P, CH], F32, tag="a_ps")
                        nc.scalar.activation(a_ps[:], delta[:], AF.Exp,
                                             scale=ac_t[d][:, n:n + 1])
                        b_sb = wp3.tile([P, CH], BF16, tag="b_sb")
                        nc.vector.tensor_tensor(out=b_sb[:], in0=du[:],
                                                in1=brep[:, nsl], op=AL.mult)
                        init = 0.0 if c2 == 0 else hlast[d][:, n:n + 1]
                        nc.vector.tensor_tensor_scan(
                            out=h_all[:, nsl], data0=a_ps[:], data1=b_sb[:],
                            initial=init, op0=AL.mult, op1=AL.add)
                    # save last state (strided copy) BEFORE overwriting h_all
                    if c2 + 1 < NCH:
                        nc.vector.tensor_copy(
                            hlast[d][:, :],
                            h_all[:, CH - 1::CH])
                    # y = sum_n C_n * h_n  (in-place mult then tree halving)
                    nc.vector.tensor_tensor(out=h_all[:], in0=h_all[:],
                                            in1=crep[:], op=AL.mult)
                    width = NS * CH // 2
                    while width >= CH:
                        nc.vector.tensor_tensor(
                            out=h_all[:, 0:width],
                            in0=h_all[:, 0:width],
                            in1=h_all[:, width:2 * width], op=AL.add)
                        width //= 2
                    # y in f16 straight into the per-chunk staging tile
                    nc.vector.scalar_tensor_tensor(
                        out=y16_all[:, d * CH:(d + 1) * CH],
                        in0=xsT_t[d][:, csl],
                        scalar=ds_t[d][:, :1], in1=h_all[:, 0:CH],
                        op0=AL.mult, op1=AL.add)

                # ---- output: transpose to token-major, int8-quantize, scatter
                for tt in range(CH // P):
                    ytd = ytdp.tile([P, DM], F16, tag="ytd")
                    tprev = None
                    for d in range(DB):
                        tr = nc.sync.dma_start_transpose(
                            out=ytd[:, d * P:(d + 1) * P],
                            in_=y16_all[:, d * CH + tt * P:d * CH + (tt + 1) * P],
                        )
                        if tprev is not None:
                            add_dep_helper(tr.ins, tprev.ins, True, "ytr chain")
                        tprev = tr
                    # per-token scale, quantize via magic add, pack bytes
                    mx = wp.tile([P, 1], F32, tag="qmx")
                    nc.vector.tensor_reduce(mx[:], ytd[:], axis=AX.X,
                                            op=AL.max)
                    mn = wp.tile([P, 1], F32, tag="qmn")
                    nc.vector.tensor_reduce(mn[:], ytd[:], axis=AX.X,
                                            op=AL.min)
                    nc.vector.tensor_scalar(out=mn[:], in0=mn[:],
                                            scalar1=-1.0, scalar2=None,
                                            op0=AL.mult)
                    nc.vector.tensor_tensor(out=mx[:], in0=mx[:], in1=mn[:],
                                            op=AL.max)
                    inv = wp.tile([P, 1], F32, tag="qinv")
                    nc.vector.reciprocal(inv[:], mx[:])
                    inv127 = wp.tile([P, 1], F32, tag="qinv127")
                    nc.vector.tensor_scalar(out=inv127[:], in0=inv[:],
                                            scalar1=127.0, scalar2=None,
                                            op0=AL.mult)
                    t32q = ytdp.tile([P, DM], F32, tag="t32q")
                    nc.vector.tensor_scalar(out=t32q[:], in0=ytd[:],
                                            scalar1=inv127[:, :1], scalar2=BIAS,
                                            op0=AL.mult, op1=AL.add)
                    qpk = ytdp.tile([P, RW], I32, tag="qpk")
                    for kk in range(4):
                        lane = qp.tile([P, DM // 4], F32, tag="pk_lane")
                        nc.vector.tensor_copy(lane[:], t32q[:, kk::4])
                        li = lane[:].bitcast(I32)
                        if kk:
                            nc.vector.tensor_scalar(
                                out=li, in0=li, scalar1=255,
                                scalar2=8 * kk, op0=AL.bitwise_and,
                                op1=AL.logical_shift_left)
                            nc.vector.tensor_tensor(
                                out=qpk[:, 0:DM // 4], in0=qpk[:, 0:DM // 4],
                                in1=li, op=AL.bitwise_or)
                        else:
                            nc.vector.tensor_scalar(
                                out=qpk[:, 0:DM // 4], in0=li,
                                scalar1=255, scalar2=None, op0=AL.bitwise_and)
                    # f16 dequant scale into byte offset 1024
                    nc.vector.tensor_scalar(
                        out=qpk[:].bitcast(F16)[:, 2 * (RW - 2):2 * (RW - 2) + 1],
                        in0=mx[:], scalar1=1.0 / 127.0, scalar2=None,
                        op0=AL.mult)
                    sc = nc.gpsimd.indirect_dma_start(
                        out=y_q[:, :],
                        out_offset=bass.IndirectOffsetOnAxis(
                            ap=sid_t[c2 * (CH // P) + tt][:, :1], axis=0),
                        in_=qpk[:],
                        in_offset=None,
                        bounds_check=L - 1,
                        oob_is_err=False,
                    )
                    if scat_prev is not None:
                        add_dep_helper(sc.ins, scat_prev.ins, True, "scat chain")
                    scat_prev = sc
    nc.compile()
    return nc


_EPS = 1e-12


def _pack_consts(means, prompt_weight, x_proj_weight, dt_projs_weight,
                 dt_projs_bias, A_logs, Ds):
    cluster_prompts = means @ prompt_weight.T          # (K, NS)
    A = -np.exp(A_logs)                                # (DM, NS)
    cb128 = np.zeros((P, 144), np.float32)
    for d in range(DB):
        cb128[:, d * NS:(d + 1) * NS] = A[d * P:(d + 1) * P, :]
        cb128[:, 128 + d] = Ds[d * P:(d + 1) * P]
        cb128[:, 136 + d] = dt_projs_bias[d * P:(d + 1) * P]
    cbb = np.zeros((DR, 1168), np.float32)
    cbb[:, 0:DM] = dt_projs_weight.T
    cbb[0:K, DM:DM + NS] = cluster_prompts
    cbb[0, DM + NS:DM + NS + P] = 1.0
    wxp80 = np.concatenate([
        x_proj_weight[0:DR],                     # dts rows 0:32
        x_proj_weight[DR + NS:DR + 2 * NS],      # Cm rows 32:48
        np.zeros((NS, DM), np.float32),          # pad rows 48:64
        x_proj_weight[DR:DR + NS],               # Bm rows 64:80
    ], axis=0).T                                 # (DM, 80)
    return {
        "cblob128": cb128,
        "cblobb": cbb.astype(BF16NP),
        "wxpT": np.ascontiguousarray(
            wxp80.reshape(DB, P, 80).transpose(1, 0, 2).reshape(P, DB * 80)
        ).astype(BF16NP),
    }


NW = 8               # waves
WC = B // NW         # cores (= batch rows) per wave


class _Runtime:
    def __init__(self):
        import jax
        from jax.sharding import Mesh, PartitionSpec, NamedSharding
        from jax.experimental.shard_map import shard_map

        self.jax = jax
        nc = build_program()
        self.nc = nc
        bass2jax.install_neuronx_cc_hook()
        # the 8 per-core executables all lower the same BIR; serialize once
        _json_cache = []
        _orig_to_json = nc.to_json_bytes

        def _to_json_cached():
            if not _json_cache:
                _json_cache.append(_orig_to_json())
            return _json_cache[0]
        nc.to_json_bytes = _to_json_cached

        partition_name = (nc.partition_id_tensor.name
                          if nc.partition_id_tensor else None)
        in_names, in_shapes, in_dtypes = [], [], []
        out_names, out_avals = [], []
        for alloc in nc.m.functions[0].allocations:
            if not isinstance(alloc, mybir.MemoryLocationSet):
                continue
            name = alloc.memorylocations[0].name
            if alloc.kind == "ExternalInput":
                if name != partition_name:
                    in_names.append(name)
                    in_shapes.append(tuple(alloc.tensor_shape))
                    in_dtypes.append(mybir.dt.np(alloc.dtype))
            elif alloc.kind == "ExternalOutput":
                out_names.append(name)
                out_avals.append(jax.core.ShapedArray(
                    tuple(alloc.tensor_shape), mybir.dt.np(alloc.dtype)))
        self.in_names = in_names
        names_cfg = list(in_names) + ([partition_name] if partition_name else [])

        def _body(*args):
            operands = list(args)
            if partition_name is not None:
                operands.append(bass2jax.partition_id_tensor())
            outs = bass2jax._bass_exec_p.bind(
                *operands,
                out_avals=tuple(out_avals),
                in_names=tuple(names_cfg),
                out_names=tuple(out_names),
                lowering_input_output_aliases=(),
                sim_require_finite=True,
                sim_require_nnan=True,
                nc=nc,
            )
            return tuple(outs)

        devices = jax.devices()[:B]
        spec = PartitionSpec("core")
        self.sh_w = []
        self.compiled_w = []
        for w in range(NW):
            mesh = Mesh(np.asarray(devices[w * WC:(w + 1) * WC]), ("core",))
            sh = NamedSharding(mesh, spec)
            self.sh_w.append(sh)
            fn = shard_map(_body, mesh=mesh,
                           in_specs=(spec,) * len(in_names),
                           out_specs=spec,
                           check_rep=False)

            def wrapped(*args, _fn=fn):
                return _fn(*args)[0]
            structs = [
                jax.ShapeDtypeStruct((WC * s[0],) + s[1:], d, sharding=sh)
                for s, d in zip(in_shapes, in_dtypes)
            ]
            self.compiled_w.append(bass2jax.fast_dispatch_compile(
                lambda: jax.jit(wrapped).lower(*structs).compile()))
        self.const_key = None
        self.const_dev = None
        dummy_w = [
            [self.put(np.zeros((WC * s[0],) + s[1:], d), w)
             for s, d in zip(in_shapes, in_dtypes)]
            for w in range(NW)
        ]
        for w in range(NW):
            jax.block_until_ready(self.compiled_w[w](*dummy_w[w]))

    def put(self, arr, w):
        return self.jax.device_put(arr, self.sh_w[w])

    def set_consts(self, weights):
        key = tuple(np.asarray(w).tobytes() for w in weights)
        if self.const_key is not None and key == self.const_key:
            return
        consts = _pack_consts(*[np.asarray(w, np.float32) for w in weights])
        self.const_dev = [
            [self.put(np.ascontiguousarray(np.tile(consts[n], (WC, 1))), w)
             for n in ("cblob128", "cblobb", "wxpT")]
            for w in range(NW)
        ]
        self.jax.block_until_ready(self.const_dev)
        self.const_key = key


_RT = None


def _runtime():
    global _RT
    if _RT is None:
        _RT = _Runtime()
    return _RT


def kernel(x, means, prompt_weight, x_proj_weight, dt_projs_weight,
           dt_projs_bias, A_logs, Ds):
    try:
        return _kernel(x, means, prompt_weight, x_proj_weight,
                       dt_projs_weight, dt_projs_bias, A_logs, Ds)
    except Exception:
        import time
        time.sleep(2.0)
        return _kernel(x, means, prompt_weight, x_proj_weight,
                       dt_projs_weight, dt_projs_bias, A_logs, Ds)


def _kernel(x, means, prompt_weight, x_proj_weight, dt_projs_weight,
            dt_projs_bias, A_logs, Ds):
    x = np.asarray(x, np.float32)
    rt = _runtime()
    rt.set_consts((means, prompt_weight, x_proj_weight, dt_projs_weight,
                   dt_projs_bias, A_logs, Ds))

    # ---- host routing: argmax over cosine scores (x-norm is k-invariant) ----
    means = np.asarray(means, np.float32)
    mnorm = means / np.maximum(
        np.linalg.norm(means, axis=-1, keepdims=True), _EPS)
    scores = x.reshape(B * L, DM) @ mnorm.T            # (B*L, K) f32 sgemm
    buckets = np.argmax(scores, axis=1).reshape(B, L)
    sidx = np.argsort(buckets, axis=1, kind="stable").astype(np.int32)  # (B,L)
    b_sorted = np.take_along_axis(buckets, sidx, axis=1)                # (B,L)
    ohs = (b_sorted[:, None, :] == np.arange(K)[None, :, None])
    ohs = ohs.astype(np.float32).astype(BF16NP)        # (B,K,L)

    # ---- wave pipeline: quantize+pack -> upload -> exec -> fetch+decode ----
    import concurrent.futures as cf
    xr = x.reshape(B * L, DM)
    y = np.empty((B, L, DM), np.float32)

    def quant_and_put(w):
        # one wave == one core == one batch row; single packed upload
        rows = slice(w * WC * L, (w + 1) * WC * L)
        xw = xr[rows]
        mx = np.maximum(xw.max(axis=1), -xw.min(axis=1))
        scl16 = (mx * (1.0 / 127.0)).astype(np.float16)
        inv127 = 1.0 / scl16.astype(np.float32)
        xq = np.empty((WC * XR, 4 * RW), np.uint8)
        xq3 = xq.reshape(WC, XR, 4 * RW)
        q = xw * inv127[:, None]
        np.rint(q, out=q)
        q += 128.0
        for c in range(WC):
            b = w * WC + c
            xq3[c, 0:L, 0:DM] = q[c * L:(c + 1) * L]
            xq3[c, 0:L, DM:DM + 2] = (
                scl16[c * L:(c + 1) * L].view(np.uint16)[:, None].view(np.uint8))
            xq3[c, 0:L, DM + 2:] = 0
            xq3[c, L:L + SX, 0:1024] = (
                sidx[b].view(np.uint8).reshape(SX, 1024))
            xq3[c, L + SX:XR, 0:1024] = (
                np.ascontiguousarray(ohs[b]).view(np.uint8).reshape(SO, 1024))
            xq3[c, L:XR, 1024:] = 0
        return rt.put(xq.view(np.int32), w)

    def fetch_decode(w, out):
        yq = np.asarray(out).view(np.uint8)             # (WC*L, 4*RW) bytes
        yv = yq[:, 0:DM].astype(np.float32)             # biased bytes (q+128)
        yv -= 128.0
        ysc = yq[:, DM:DM + 2].copy().view(np.float16).astype(np.float32)
        yv *= ysc
        y[w * WC:(w + 1) * WC] = yv.reshape(WC, L, DM)

    with cf.ThreadPoolExecutor(2 * NW) as ex:
        put_futs = [ex.submit(quant_and_put, w) for w in range(NW)]
        fetch_futs = []
        for w, f in enumerate(put_futs):
            out = rt.compiled_w[w](f.result(), *rt.const_dev[w])
            fetch_futs.append(ex.submit(fetch_decode, w, out))
        for f in fetch_futs:
            f.result()
    return y


if __name__ == "__main__":
    np.random.seed(0)
    ins = {
        "x": np.random.randn(B, L, DM).astype(np.float32),
        "means": np.random.randn(K, DM).astype(np.float32),
        "prompt_weight": np.random.randn(NS, DM).astype(np.float32) * DM ** -0.5,
        "x_proj_weight": np.random.randn(DR + 2 * NS, DM).astype(np.float32) * DM ** -0.5,
        "dt_projs_weight": np.random.uniform(-DR ** -0.5, DR ** -0.5, (DM, DR)).astype(np.float32),
        "dt_projs_bias": np.random.randn(DM).astype(np.float32),
        "A_logs": np.log(np.broadcast_to(np.arange(1, NS + 1, dtype=np.float32), (DM, NS))).copy(),
        "Ds": np.ones(DM, np.float32),
    }
    import time
    o = kernel(**ins)
    t0 = time.time()
    o = kernel(**ins)
    print(f"warm call: {time.time()-t0:.2f}s")
    print("ok", o.shape, o.dtype)
